# revision 51
# baseline (speedup 1.0000x reference)
"""Distributed Trainium2 kernel for the dense-graph GNN layer.

Math: with xn = x/||x|| (rows), G = xn@xn.T, d = rsqrt(G@1),
out = (diag(d) G diag(d) x) W.  The N x N Gram matrix is never needed:
  G @ 1        = xn @ t,            t = colsum(xn)            [D]
  diag(d) G diag(d) x = f * (x @ z),  z = x.T @ diag(f) @ x   [D, D]
  f_i = d_i / ||x_i||   (combines both scalings; z is symmetric)
  out = f * (x @ (z @ W))
Each core processes its 1024-row shard; the only cross-core traffic is
(1) a reduction of the [D] colsum partial and (2) a reduction of the
[D, D] (z @ W) partial.  Both are done with direct peer-to-peer SBUF
remote DMAs (XOR-slotted mesh exchange + local slot-sum) instead of
ncfw collectives, which removes the ~10us-per-op collective floor and
the serial entry barrier from the critical path.  A 1-byte prelude
AllGather (bir_kernel_barrier_wait) guarantees every peer has entered
the kernel (and cleared its semaphores) before any remote write fires;
it runs concurrently with phase A.
"""

import os
import sys

import numpy as np

for _p in ("/opt/trn_rl_repo", "/root/.axon_site/_ro/trn_rl_repo"):
    if os.path.isdir(_p) and _p not in sys.path:
        sys.path.insert(0, _p)

import concourse.bacc as bacc
import concourse.mybir as mybir
import concourse.tile as tile
import concourse.masks as masks
from concourse import bass_utils

R = 8                 # cores
N, D = 8192, 256
NL = N // R           # 1024 rows per core
P = 128
T = NL // P           # 8 row tiles per core
F32 = mybir.dt.float32
BF16 = mybir.dt.bfloat16
AF = mybir.ActivationFunctionType
ALU = mybir.AluOpType

TSLOT = 8             # padded t-slot width (f32 cols) = 32 B per partition
ZSLOT = 2 * D         # zw-slot width (bf16 cols)  = 1 KB per partition

_cache = {}


def _program(tc, x, W, out):
    nc = tc.nc
    rsem_t = nc.alloc_semaphore("rsem_t")
    rsem_zw = nc.alloc_semaphore("rsem_zw")
    lsem = nc.alloc_semaphore("rdma_local")
    lsem2 = nc.alloc_semaphore("rdma_local_q1")
    with (
        tc.tile_pool(name="persist", bufs=1) as pp,
        tc.tile_pool(name="work", bufs=3) as wp,
        tc.tile_pool(name="psum", bufs=1, space="PSUM") as psp,
        tc.tile_pool(name="psumw", bufs=4, space="PSUM") as psw,
    ):
        x_all = pp.tile([P, T * D], F32)      # row tile i at [:, i*D:(i+1)*D]
        xb_all = pp.tile([P, T * D], BF16)    # bf16 copy of x
        g_all = pp.tile([P, T * D], BF16)     # f * x (bf16)
        xT_all = pp.tile([P, 2 * NL], BF16)   # x.T chunk c at [:, c*NL + i*P]
        W_sb = pp.tile([P, 2 * D], F32)       # W k-chunk kc at [:, kc*D]
        Wb_sb = pp.tile([P, 2 * D], BF16)
        zT_sb = pp.tile([P, 2 * D], BF16)

        ss = pp.tile([P, T], F32)
        invn = pp.tile([P, T], F32)
        nrm = pp.tile([P, T], F32)
        stl = pp.tile([P, T], F32)
        s_t = pp.tile([P, T], F32)
        sq_s = pp.tile([P, T], F32)
        dd = pp.tile([P, T], F32)
        f_t = pp.tile([P, T], F32)

        ident = pp.tile([P, P], F32)
        masks.make_identity(nc, ident[:])
        ones2 = pp.tile([2, P], F32)
        nc.gpsimd.memset(ones2[:], 1.0)

        # remote-exchange buffers (slot d of the gather buffers is written
        # by the peer whose physical id is mine^d; slot 0 is the loopback).
        t_col = pp.tile([P, TSLOT], F32)      # my colsum partial, cols 0-1
        tgath = pp.tile([P, 8 * TSLOT], F32)
        zw_loc = pp.tile([P, ZSLOT], BF16)    # my (z @ W) partial
        zwg = pp.tile([P, 8 * ZSLOT], BF16)
        t_row = pp.tile([1, 2 * P], F32)
        tb_sb = pp.tile([P, D], F32)

        nc.gpsimd.memset(t_col[:], 0.0)

        # One broadcast per XOR distance dq, all lanes carrying the same
        # destination (peer mine^dq) so every descriptor moves real data —
        # a None slot would emit dummy descriptors that flood the SDMA
        # engines.  Same-die dests (dq<4) may use all 16 lanes (+16 on the
        # receiver's sem); cross-die dests are restricted to the 8
        # D2D-capable lanes (+8).  Arrival total: 4*16 + 4*8 = 96.
        def _rdests(dq):
            if dq < 4:
                return [(0, dq)] * 8
            rd = [None] * 16
            for s in (4, 5, 6, 7, 12, 13, 14, 15):
                rd[s] = (0, dq)
            return rd

        # Round-1 send descriptors (desc-gen only; the DMAs read t_col when
        # trigger_dma fires).
        for dq in range(R):
            nc.gpsimd.remote_dma_broadcast(
                tgath[:, dq * TSLOT:(dq + 1) * TSLOT], t_col[:],
                rsem_t, lsem, rdests=_rdests(dq),
            )

        for kc in range(2):
            nc.sync.dma_start(W_sb[:, kc * D:(kc + 1) * D], W[kc * P:(kc + 1) * P, :])
        nc.vector.tensor_copy(Wb_sb[:], W_sb[:])

        # ---- phase A: load shard, row norms, colsum(xn) partial ----
        for i in range(T):
            xs = x_all[:, i * D:(i + 1) * D]
            nc.sync.dma_start(xs, x[i * P:(i + 1) * P, :])
            scr = wp.tile([P, D], F32, tag="scr", name=f"scr{i}")
            nc.scalar.activation(scr[:], xs, AF.Square, accum_out=ss[:, i:i + 1])
            nc.vector.tensor_copy(xb_all[:, i * D:(i + 1) * D], xs)
        nc.scalar.activation(nrm[:], ss[:], AF.Sqrt)
        nc.vector.reciprocal(invn[:], nrm[:])

        # t partial in column layout: t_col[p, c] = sum_i (x_i chunk c)^T invn_i
        psum_tc = psw.tile([P, D], F32, tag="pw", name="psum_tc")
        for c in range(2):
            for i in range(T):
                nc.tensor.matmul(
                    psum_tc[:, c:c + 1],
                    lhsT=x_all[:, i * D + c * P:i * D + (c + 1) * P],
                    rhs=invn[:, i:i + 1],
                    start=(i == 0), stop=(i == T - 1),
                )
        nc.vector.tensor_copy(t_col[:, 0:2], psum_tc[:, 0:2])

        # x.T via PE transposes (independent work that overlaps the exchange)
        for i in range(T):
            for c in range(2):
                pt = psw.tile([P, P], F32, tag="pw", name=f"pt{i}_{c}")
                nc.tensor.transpose(
                    pt[:], x_all[:, i * D + c * P: i * D + (c + 1) * P], ident[:]
                )
                nc.vector.tensor_copy(xT_all[:, c * NL + i * P: c * NL + (i + 1) * P], pt[:])

        # Fire round 1.  No entry barrier is needed: target_bir_lowering is
        # off, so there is no per-kernel sem clear — sems are zeroed at NEFF
        # load and remote increments persist even if a peer has not started
        # executing yet.  signals_writable declares the trigger as a writer
        # of t_col (orders it after the producer — the preps predate the
        # producer so the deferred-RAW edge does not form) and of tgath
        # (orders the slot-sum consumers after the trigger).
        trig1 = nc.gpsimd.trigger_dma(count=None, signals_writable=(t_col[:], tgath[:]))

        # Round-2 send descriptors while round 1 is in flight (their own
        # SWDGE queue — the Tile scheduler may interleave Pool instructions
        # across rounds, which must not mix the trigger FIFOs).
        for dq in range(R):
            nc.gpsimd.remote_dma_broadcast(
                zwg[:, dq * ZSLOT:(dq + 1) * ZSLOT], zw_loc[:],
                rsem_zw, lsem2, rdests=_rdests(dq), queue_num=1,
            )

        # ---- phase B: degrees, f, g = f*x, zT partial, zw partial ----
        # Sum the 8 t slots; the first add carries the all-arrivals wait
        # (4 same-die sends x 16 + 4 cross-die x 8 = 96), attached
        # post-schedule.  The trace-time lsem wait (my own 8 sends drained,
        # 8 x 16) is a schedule-visible stand-in that keeps the add from
        # being placed before the trigger in the DVE queue — without it the
        # engine would stall on the cross-core wait before doing the work
        # that feeds the trigger.
        add_t = nc.vector.tensor_add(
            tgath[:, 0:4 * TSLOT], tgath[:, 0:4 * TSLOT], tgath[:, 4 * TSLOT:8 * TSLOT]
        )
        nc.vector.tensor_add(
            tgath[:, 0:2 * TSLOT], tgath[:, 0:2 * TSLOT], tgath[:, 2 * TSLOT:4 * TSLOT]
        )
        nc.vector.tensor_add(
            tgath[:, 0:TSLOT], tgath[:, 0:TSLOT], tgath[:, TSLOT:2 * TSLOT]
        )

        # t (column layout) -> per-chunk row at partition 0 -> broadcast to
        # 128 partitions via K=1 ones-matmuls (everything base-partition 0).
        psum_tb = psw.tile([P, D], F32, tag="pw", name="psum_tb")
        for c in range(2):
            ptr = psw.tile([P, D], F32, tag="pw", name=f"ptr{c}")
            nc.tensor.transpose(ptr[0:1, 0:P], tgath[:, c:c + 1], ident[:])
            nc.vector.tensor_copy(t_row[0:1, c * P:(c + 1) * P], ptr[0:1, 0:P])
            nc.tensor.matmul(
                psum_tb[:, c * P:(c + 1) * P],
                lhsT=ones2[0:1, :],
                rhs=t_row[0:1, c * P:(c + 1) * P],
                start=True, stop=True,
            )
        tb_copy = nc.vector.tensor_copy(tb_sb[:], psum_tb[:])

        big_scr = pp.tile([P, T * D], F32)
        t_ap = tb_sb[:]
        from concourse.bass_types import AP as _AP
        t_rep = _AP(t_ap.tensor, t_ap.offset, [t_ap.ap[0], [0, T], t_ap.ap[1]])
        x3 = x_all[:].rearrange("p (t d) -> p t d", t=T)
        s3 = big_scr[:].rearrange("p (t d) -> p t d", t=T)
        nc.vector.tensor_mul(s3, x3, t_rep)
        nc.vector.tensor_reduce(stl[:], s3, axis=mybir.AxisListType.X, op=ALU.add)
        nc.vector.tensor_mul(s_t[:], stl[:], invn[:])       # s = rowsum * invn
        nc.scalar.activation(sq_s[:], s_t[:], AF.Sqrt)
        nc.vector.reciprocal(dd[:], sq_s[:])                # d = rsqrt(s)
        nc.vector.tensor_mul(f_t[:], dd[:], invn[:])        # f = d * invn
        for i in range(T):
            nc.scalar.mul(g_all[:, i * D:(i + 1) * D], x_all[:, i * D:(i + 1) * D],
                          f_t[:, i:i + 1])

        psum_zT0 = psp.tile([P, D], F32, name="pzT0")
        psum_zT1 = psp.tile([P, D], F32, name="pzT1")
        for i in range(T):
            for c, pz in ((0, psum_zT0), (1, psum_zT1)):
                nc.tensor.matmul(
                    pz[:], lhsT=xb_all[:, i * D + c * P: i * D + (c + 1) * P],
                    rhs=g_all[:, i * D:(i + 1) * D],
                    start=(i == 0), stop=(i == T - 1),
                )
        for c, pz in ((0, psum_zT0), (1, psum_zT1)):
            nc.vector.tensor_copy(zT_sb[:, c * D:(c + 1) * D], pz[:])

        # zw partial = z_p @ W (fold the W GEMM before the exchange)
        for m in range(2):
            pzw = psw.tile([P, D], F32, tag="pw", name=f"pzw{m}")
            for kc in range(2):
                nc.tensor.matmul(
                    pzw[:], lhsT=zT_sb[:, kc * D + m * P: kc * D + (m + 1) * P],
                    rhs=Wb_sb[:, kc * D:(kc + 1) * D],
                    start=(kc == 0), stop=(kc == 1),
                )
            nc.vector.tensor_copy(zw_loc[:, m * D:(m + 1) * D], pzw[:])

        # Fire round 2 (same signals_writable trick: after both zw_loc
        # casts, before the zwg slot-sums).
        trig2 = nc.gpsimd.trigger_dma(count=None, queue_num=1,
                                      signals_writable=(zw_loc[:], zwg[:]))

        # ---- phase C: out = f * (x @ zw) ----
        add_zw = nc.vector.tensor_add(
            zwg[:, 0:4 * ZSLOT], zwg[:, 0:4 * ZSLOT], zwg[:, 4 * ZSLOT:8 * ZSLOT]
        )
        nc.vector.tensor_add(
            zwg[:, 0:2 * ZSLOT], zwg[:, 0:2 * ZSLOT], zwg[:, 2 * ZSLOT:4 * ZSLOT]
        )
        nc.vector.tensor_add(
            zwg[:, 0:ZSLOT], zwg[:, 0:ZSLOT], zwg[:, ZSLOT:2 * ZSLOT]
        )

        for i in range(T):
            po = psw.tile([P, D], F32, tag="pw", name=f"po{i}")
            for ka in range(2):
                nc.tensor.matmul(
                    po[:], lhsT=xT_all[:, ka * NL + i * P: ka * NL + (i + 1) * P],
                    rhs=zwg[:, ka * D:(ka + 1) * D],
                    start=(ka == 0), stop=(ka == 1),
                )
            o_sb = wp.tile([P, D], F32, tag="osb", name=f"osb{i}")
            nc.scalar.mul(o_sb[:], po[:], f_t[:, i:i + 1])
            nc.sync.dma_start(out[i * P:(i + 1) * P, :], o_sb[:])


    return {"add_t": add_t, "add_zw": add_zw,
            "rsem_t": rsem_t, "rsem_zw": rsem_zw}


def _build():
    nc = bacc.Bacc("TRN2", target_bir_lowering=False, debug=False, num_devices=R,
                   num_swdge_queues=2)
    x = nc.dram_tensor("x", [NL, D], F32, kind="ExternalInput")
    W = nc.dram_tensor("W", [D, D], F32, kind="ExternalInput")
    out = nc.dram_tensor("out", [NL, D], F32, kind="ExternalOutput")
    with tile.TileContext(nc) as tc:
        h = _program(tc, x.ap() if hasattr(x, "ap") else x, W.ap() if hasattr(W, "ap") else W, out.ap() if hasattr(out, "ap") else out)
    # Attach the cross-core waits after scheduling (the schedule-time
    # single-core sim cannot model peer sem increments, and added waits
    # only delay — they cannot invalidate the schedule).  compile()
    # splits multi-wait instructions into event semaphores automatically.
    h["add_t"].wait_op(h["rsem_t"], 96, "sem-ge", check=False)
    h["add_zw"].wait_op(h["rsem_zw"], 96, "sem-ge", check=False)
    nc.finalize()
    return nc


def _run(inputs, trace=False):
    if "nc" not in _cache:
        _cache["nc"] = _build()
    nc = _cache["nc"]
    x = np.ascontiguousarray(inputs["x"], dtype=np.float32)
    W = np.ascontiguousarray(inputs["W"], dtype=np.float32)
    in_maps = [{"x": x[r * NL:(r + 1) * NL], "W": W} for r in range(R)]
    res = bass_utils.run_bass_kernel_spmd(
        nc, in_maps, core_ids=list(range(R)), trace=trace,
    )
    out = np.concatenate([res.results[r]["out"] for r in range(R)], axis=0)
    return out, res


def kernel(**inputs) -> np.ndarray:
    out, _ = _run(inputs, trace=False)
    return out


# revision 52
# speedup vs baseline: 34.6056x; 34.6056x over previous
"""Distributed Trainium2 kernel for the dense-graph GNN layer.

Math: with xn = x/||x|| (rows), G = xn@xn.T, d = rsqrt(G@1),
out = (diag(d) G diag(d) x) W.  The N x N Gram matrix is never needed:
  G @ 1        = xn @ t,            t = colsum(xn)            [D]
  diag(d) G diag(d) x = f * (x @ z),  z = x.T @ diag(f) @ x   [D, D]
  f_i = d_i / ||x_i||   (combines both scalings; z is symmetric)
  out = f * (x @ (z @ W))
Each core processes its 1024-row shard; the only cross-core traffic is
(1) a reduction of the [D] colsum partial and (2) a reduction of the
[D, D] (z @ W) partial.  Both are done with direct peer-to-peer SBUF
remote DMAs (XOR-slotted mesh exchange + local slot-sum) instead of
ncfw collectives, which removes the ~10us-per-op collective floor and
the serial entry barrier from the critical path.  A 1-byte prelude
AllGather (bir_kernel_barrier_wait) guarantees every peer has entered
the kernel (and cleared its semaphores) before any remote write fires;
it runs concurrently with phase A.
"""

import os
import sys

import numpy as np

for _p in ("/opt/trn_rl_repo", "/root/.axon_site/_ro/trn_rl_repo"):
    if os.path.isdir(_p) and _p not in sys.path:
        sys.path.insert(0, _p)

import concourse.bacc as bacc
import concourse.mybir as mybir
import concourse.tile as tile
import concourse.masks as masks
from concourse import bass_utils

R = 8                 # cores
N, D = 8192, 256
NL = N // R           # 1024 rows per core
P = 128
T = NL // P           # 8 row tiles per core
F32 = mybir.dt.float32
BF16 = mybir.dt.bfloat16
AF = mybir.ActivationFunctionType
ALU = mybir.AluOpType

TSLOT = 8             # padded t-slot width (f32 cols) = 32 B per partition
ZSLOT = 2 * D         # zw-slot width (bf16 cols)  = 1 KB per partition

_cache = {}


def _program(tc, x, W, out):
    nc = tc.nc
    rsem_t = nc.alloc_semaphore("rsem_t")
    rsem_zw = nc.alloc_semaphore("rsem_zw")
    lsem = nc.alloc_semaphore("rdma_local")
    lsem2 = nc.alloc_semaphore("rdma_local_q1")
    with (
        tc.tile_pool(name="persist", bufs=1) as pp,
        tc.tile_pool(name="work", bufs=3) as wp,
        tc.tile_pool(name="psum", bufs=1, space="PSUM") as psp,
        tc.tile_pool(name="psumw", bufs=4, space="PSUM") as psw,
        tc.tile_pool(name="dram", bufs=1, space="DRAM") as dp,
    ):
        # A dangling 1-byte AllGather nothing waits on: its only purpose is
        # to mark the NEFF as having collectives so the runtime performs a
        # coordinated (rendezvous) launch across the 8 cores — without it,
        # per-core dispatch is staggered by milliseconds, which the remote
        # DMA waits would absorb into the measured span.
        cc_in = dp.tile([1, 1], F32)
        cc_out = dp.tile([R, 1], F32)
        nc.gpsimd.collective_compute(
            "AllGather", ALU.bypass, replica_groups=[list(range(R))],
            ins=[cc_in.opt()], outs=[cc_out.opt()],
        )
        x_all = pp.tile([P, T * D], F32)      # row tile i at [:, i*D:(i+1)*D]
        xb_all = pp.tile([P, T * D], BF16)    # bf16 copy of x
        g_all = pp.tile([P, T * D], BF16)     # f * x (bf16)
        xT_all = pp.tile([P, 2 * NL], BF16)   # x.T chunk c at [:, c*NL + i*P]
        W_sb = pp.tile([P, 2 * D], F32)       # W k-chunk kc at [:, kc*D]
        Wb_sb = pp.tile([P, 2 * D], BF16)
        zT_sb = pp.tile([P, 2 * D], BF16)

        ss = pp.tile([P, T], F32)
        invn = pp.tile([P, T], F32)
        nrm = pp.tile([P, T], F32)
        stl = pp.tile([P, T], F32)
        s_t = pp.tile([P, T], F32)
        sq_s = pp.tile([P, T], F32)
        dd = pp.tile([P, T], F32)
        f_t = pp.tile([P, T], F32)

        ident = pp.tile([P, P], F32)
        masks.make_identity(nc, ident[:])
        ones2 = pp.tile([2, P], F32)
        nc.gpsimd.memset(ones2[:], 1.0)

        # remote-exchange buffers (slot d of the gather buffers is written
        # by the peer whose physical id is mine^d; slot 0 is the loopback).
        t_col = pp.tile([P, TSLOT], F32)      # my colsum partial, cols 0-1
        tgath = pp.tile([P, 8 * TSLOT], F32)
        zw_loc = pp.tile([P, ZSLOT], BF16)    # my (z @ W) partial
        zwg = pp.tile([P, 8 * ZSLOT], BF16)
        t_row = pp.tile([1, 2 * P], F32)
        tb_sb = pp.tile([P, D], F32)

        nc.gpsimd.memset(t_col[:], 0.0)

        # One broadcast per XOR distance dq, all lanes carrying the same
        # destination (peer mine^dq) so every descriptor moves real data —
        # a None slot would emit dummy descriptors that flood the SDMA
        # engines.  Same-die dests (dq<4) may use all 16 lanes (+16 on the
        # receiver's sem); cross-die dests are restricted to the 8
        # D2D-capable lanes (+8).  Arrival total: 4*16 + 4*8 = 96.
        def _rdests(dq):
            if dq < 4:
                return [(0, dq)] * 8
            rd = [None] * 16
            for s in (4, 5, 6, 7, 12, 13, 14, 15):
                rd[s] = (0, dq)
            return rd

        # Round-1 send descriptors (desc-gen only; the DMAs read t_col when
        # trigger_dma fires).
        for dq in range(R):
            nc.gpsimd.remote_dma_broadcast(
                tgath[:, dq * TSLOT:(dq + 1) * TSLOT], t_col[:],
                rsem_t, lsem, rdests=_rdests(dq),
            )

        for kc in range(2):
            nc.sync.dma_start(W_sb[:, kc * D:(kc + 1) * D], W[kc * P:(kc + 1) * P, :])
        nc.vector.tensor_copy(Wb_sb[:], W_sb[:])

        # ---- phase A: load shard, row norms, colsum(xn) partial ----
        for i in range(T):
            xs = x_all[:, i * D:(i + 1) * D]
            nc.sync.dma_start(xs, x[i * P:(i + 1) * P, :])
            scr = wp.tile([P, D], F32, tag="scr", name=f"scr{i}")
            nc.scalar.activation(scr[:], xs, AF.Square, accum_out=ss[:, i:i + 1])
            nc.vector.tensor_copy(xb_all[:, i * D:(i + 1) * D], xs)
        nc.scalar.activation(nrm[:], ss[:], AF.Sqrt)
        nc.vector.reciprocal(invn[:], nrm[:])

        # t partial in column layout: t_col[p, c] = sum_i (x_i chunk c)^T invn_i
        psum_tc = psw.tile([P, D], F32, tag="pw", name="psum_tc")
        for c in range(2):
            for i in range(T):
                nc.tensor.matmul(
                    psum_tc[:, c:c + 1],
                    lhsT=x_all[:, i * D + c * P:i * D + (c + 1) * P],
                    rhs=invn[:, i:i + 1],
                    start=(i == 0), stop=(i == T - 1),
                )
        nc.vector.tensor_copy(t_col[:, 0:2], psum_tc[:, 0:2])

        # x.T via PE transposes (independent work that overlaps the exchange)
        for i in range(T):
            for c in range(2):
                pt = psw.tile([P, P], F32, tag="pw", name=f"pt{i}_{c}")
                nc.tensor.transpose(
                    pt[:], x_all[:, i * D + c * P: i * D + (c + 1) * P], ident[:]
                )
                nc.vector.tensor_copy(xT_all[:, c * NL + i * P: c * NL + (i + 1) * P], pt[:])

        # Fire round 1.  No entry barrier is needed: target_bir_lowering is
        # off, so there is no per-kernel sem clear — sems are zeroed at NEFF
        # load and remote increments persist even if a peer has not started
        # executing yet.  signals_writable declares the trigger as a writer
        # of t_col (orders it after the producer — the preps predate the
        # producer so the deferred-RAW edge does not form) and of tgath
        # (orders the slot-sum consumers after the trigger).
        trig1 = nc.gpsimd.trigger_dma(count=None, signals_writable=(t_col[:], tgath[:]))

        # Round-2 send descriptors while round 1 is in flight (their own
        # SWDGE queue — the Tile scheduler may interleave Pool instructions
        # across rounds, which must not mix the trigger FIFOs).
        for dq in range(R):
            nc.gpsimd.remote_dma_broadcast(
                zwg[:, dq * ZSLOT:(dq + 1) * ZSLOT], zw_loc[:],
                rsem_zw, lsem2, rdests=_rdests(dq), queue_num=1,
            )

        # ---- phase B: degrees, f, g = f*x, zT partial, zw partial ----
        # Sum the 8 t slots; the first add carries the all-arrivals wait
        # (4 same-die sends x 16 + 4 cross-die x 8 = 96), attached
        # post-schedule.  The trace-time lsem wait (my own 8 sends drained,
        # 8 x 16) is a schedule-visible stand-in that keeps the add from
        # being placed before the trigger in the DVE queue — without it the
        # engine would stall on the cross-core wait before doing the work
        # that feeds the trigger.
        add_t = nc.vector.tensor_add(
            tgath[:, 0:4 * TSLOT], tgath[:, 0:4 * TSLOT], tgath[:, 4 * TSLOT:8 * TSLOT]
        )
        nc.vector.tensor_add(
            tgath[:, 0:2 * TSLOT], tgath[:, 0:2 * TSLOT], tgath[:, 2 * TSLOT:4 * TSLOT]
        )
        nc.vector.tensor_add(
            tgath[:, 0:TSLOT], tgath[:, 0:TSLOT], tgath[:, TSLOT:2 * TSLOT]
        )

        # t (column layout) -> per-chunk row at partition 0 -> broadcast to
        # 128 partitions via K=1 ones-matmuls (everything base-partition 0).
        psum_tb = psw.tile([P, D], F32, tag="pw", name="psum_tb")
        for c in range(2):
            ptr = psw.tile([P, D], F32, tag="pw", name=f"ptr{c}")
            nc.tensor.transpose(ptr[0:1, 0:P], tgath[:, c:c + 1], ident[:])
            nc.vector.tensor_copy(t_row[0:1, c * P:(c + 1) * P], ptr[0:1, 0:P])
            nc.tensor.matmul(
                psum_tb[:, c * P:(c + 1) * P],
                lhsT=ones2[0:1, :],
                rhs=t_row[0:1, c * P:(c + 1) * P],
                start=True, stop=True,
            )
        tb_copy = nc.vector.tensor_copy(tb_sb[:], psum_tb[:])

        big_scr = pp.tile([P, T * D], F32)
        t_ap = tb_sb[:]
        from concourse.bass_types import AP as _AP
        t_rep = _AP(t_ap.tensor, t_ap.offset, [t_ap.ap[0], [0, T], t_ap.ap[1]])
        x3 = x_all[:].rearrange("p (t d) -> p t d", t=T)
        s3 = big_scr[:].rearrange("p (t d) -> p t d", t=T)
        nc.vector.tensor_mul(s3, x3, t_rep)
        nc.vector.tensor_reduce(stl[:], s3, axis=mybir.AxisListType.X, op=ALU.add)
        nc.vector.tensor_mul(s_t[:], stl[:], invn[:])       # s = rowsum * invn
        nc.scalar.activation(sq_s[:], s_t[:], AF.Sqrt)
        nc.vector.reciprocal(dd[:], sq_s[:])                # d = rsqrt(s)
        nc.vector.tensor_mul(f_t[:], dd[:], invn[:])        # f = d * invn
        for i in range(T):
            nc.scalar.mul(g_all[:, i * D:(i + 1) * D], x_all[:, i * D:(i + 1) * D],
                          f_t[:, i:i + 1])

        psum_zT0 = psp.tile([P, D], F32, name="pzT0")
        psum_zT1 = psp.tile([P, D], F32, name="pzT1")
        for i in range(T):
            for c, pz in ((0, psum_zT0), (1, psum_zT1)):
                nc.tensor.matmul(
                    pz[:], lhsT=xb_all[:, i * D + c * P: i * D + (c + 1) * P],
                    rhs=g_all[:, i * D:(i + 1) * D],
                    start=(i == 0), stop=(i == T - 1),
                )
        for c, pz in ((0, psum_zT0), (1, psum_zT1)):
            nc.vector.tensor_copy(zT_sb[:, c * D:(c + 1) * D], pz[:])

        # zw partial = z_p @ W (fold the W GEMM before the exchange)
        for m in range(2):
            pzw = psw.tile([P, D], F32, tag="pw", name=f"pzw{m}")
            for kc in range(2):
                nc.tensor.matmul(
                    pzw[:], lhsT=zT_sb[:, kc * D + m * P: kc * D + (m + 1) * P],
                    rhs=Wb_sb[:, kc * D:(kc + 1) * D],
                    start=(kc == 0), stop=(kc == 1),
                )
            nc.vector.tensor_copy(zw_loc[:, m * D:(m + 1) * D], pzw[:])

        # Fire round 2 (same signals_writable trick: after both zw_loc
        # casts, before the zwg slot-sums).
        trig2 = nc.gpsimd.trigger_dma(count=None, queue_num=1,
                                      signals_writable=(zw_loc[:], zwg[:]))

        # ---- phase C: out = f * (x @ zw) ----
        add_zw = nc.vector.tensor_add(
            zwg[:, 0:4 * ZSLOT], zwg[:, 0:4 * ZSLOT], zwg[:, 4 * ZSLOT:8 * ZSLOT]
        )
        nc.vector.tensor_add(
            zwg[:, 0:2 * ZSLOT], zwg[:, 0:2 * ZSLOT], zwg[:, 2 * ZSLOT:4 * ZSLOT]
        )
        nc.vector.tensor_add(
            zwg[:, 0:ZSLOT], zwg[:, 0:ZSLOT], zwg[:, ZSLOT:2 * ZSLOT]
        )

        for i in range(T):
            po = psw.tile([P, D], F32, tag="pw", name=f"po{i}")
            for ka in range(2):
                nc.tensor.matmul(
                    po[:], lhsT=xT_all[:, ka * NL + i * P: ka * NL + (i + 1) * P],
                    rhs=zwg[:, ka * D:(ka + 1) * D],
                    start=(ka == 0), stop=(ka == 1),
                )
            o_sb = wp.tile([P, D], F32, tag="osb", name=f"osb{i}")
            nc.scalar.mul(o_sb[:], po[:], f_t[:, i:i + 1])
            nc.sync.dma_start(out[i * P:(i + 1) * P, :], o_sb[:])


    return {"add_t": add_t, "add_zw": add_zw,
            "rsem_t": rsem_t, "rsem_zw": rsem_zw}


def _build():
    nc = bacc.Bacc("TRN2", target_bir_lowering=False, debug=False, num_devices=R,
                   num_swdge_queues=2)
    x = nc.dram_tensor("x", [NL, D], F32, kind="ExternalInput")
    W = nc.dram_tensor("W", [D, D], F32, kind="ExternalInput")
    out = nc.dram_tensor("out", [NL, D], F32, kind="ExternalOutput")
    with tile.TileContext(nc) as tc:
        h = _program(tc, x.ap() if hasattr(x, "ap") else x, W.ap() if hasattr(W, "ap") else W, out.ap() if hasattr(out, "ap") else out)
    # Attach the cross-core waits after scheduling (the schedule-time
    # single-core sim cannot model peer sem increments, and added waits
    # only delay — they cannot invalidate the schedule).  compile()
    # splits multi-wait instructions into event semaphores automatically.
    h["add_t"].wait_op(h["rsem_t"], 96, "sem-ge", check=False)
    h["add_zw"].wait_op(h["rsem_zw"], 96, "sem-ge", check=False)
    nc.finalize()
    return nc


def _run(inputs, trace=False):
    if "nc" not in _cache:
        _cache["nc"] = _build()
    nc = _cache["nc"]
    x = np.ascontiguousarray(inputs["x"], dtype=np.float32)
    W = np.ascontiguousarray(inputs["W"], dtype=np.float32)
    in_maps = [{"x": x[r * NL:(r + 1) * NL], "W": W} for r in range(R)]
    res = bass_utils.run_bass_kernel_spmd(
        nc, in_maps, core_ids=list(range(R)), trace=trace,
    )
    out = np.concatenate([res.results[r]["out"] for r in range(R)], axis=0)
    return out, res


def kernel(**inputs) -> np.ndarray:
    out, _ = _run(inputs, trace=False)
    return out


# revision 57
# speedup vs baseline: 48.9061x; 1.4132x over previous
"""Distributed Trainium2 kernel for the dense-graph GNN layer.

Math: with xn = x/||x|| (rows), G = xn@xn.T, d = rsqrt(G@1),
out = (diag(d) G diag(d) x) W.  The N x N Gram matrix is never needed:
  G @ 1        = xn @ t,            t = colsum(xn)            [D]
  diag(d) G diag(d) x = f * (x @ z),  z = x.T @ diag(f) @ x   [D, D]
  f_i = d_i / ||x_i||   (combines both scalings; z is symmetric)
  out = f * (x @ (z @ W))
Each core processes its 1024-row shard; the only cross-core traffic is
(1) a reduction of the [D] colsum partial and (2) a reduction of the
[D, D] (z @ W) partial.  Both reductions run as recursive-doubling
exchanges over direct peer-to-peer SBUF remote DMAs (XOR partners 1, 2,
4), which avoids both the ~70us ncfw collective bringup and the
descriptor flood of a full mesh.  A dangling 1-byte AllGather marks the
NEFF as collective so the runtime gang-launches the 8 cores (without
it, dispatch is staggered by milliseconds).
"""

import os
import sys

import numpy as np

for _p in ("/opt/trn_rl_repo", "/root/.axon_site/_ro/trn_rl_repo"):
    if os.path.isdir(_p) and _p not in sys.path:
        sys.path.insert(0, _p)

import concourse.bacc as bacc
import concourse.mybir as mybir
import concourse.tile as tile
import concourse.masks as masks
from concourse import bass_utils

R = 8                 # cores
N, D = 8192, 256
NL = N // R           # 1024 rows per core
P = 128
T = NL // P           # 8 row tiles per core
F32 = mybir.dt.float32
BF16 = mybir.dt.bfloat16
AF = mybir.ActivationFunctionType
ALU = mybir.AluOpType

TSLOT = 8             # t exchange payload width (f32 cols) = 32 B/partition
ZSLOT = 2 * D         # zw exchange payload width (bf16 cols) = 1 KB/partition
HOPS = (1, 2, 4)      # recursive-doubling XOR distances

_cache = {}


def _rdests(dq):
    """8-slot dest list with the single real dest at slot dq (lanes dq and
    dq+8).  Slot index == XOR distance keeps cross-die dests (bit 2 set) on
    D2D-capable lanes.  One real slot = one copy of the payload; the dummy
    slots cost ~66 4-byte descriptors per lane but avoid the 8x payload
    amplification of repeating the dest in every slot."""
    rd = [None] * 8
    rd[dq] = (0, dq)
    return rd


def _program(tc, x, W, out):
    nc = tc.nc
    # Per-hop arrival sems (a shared counter would be ambiguous: a fast
    # partner's hop-2 arrival must not satisfy a hop-1 wait).
    rsem_t = [nc.alloc_semaphore(f"rsem_t{k}") for k in range(3)]
    rsem_z = [nc.alloc_semaphore(f"rsem_z{k}") for k in range(3)]
    # Local (send-drained) sems, one per SWDGE queue.
    lsem = [nc.alloc_semaphore(f"lsem_q{q}") for q in range(4)]
    with (
        tc.tile_pool(name="persist", bufs=1) as pp,
        tc.tile_pool(name="work", bufs=3) as wp,
        tc.tile_pool(name="psum", bufs=1, space="PSUM") as psp,
        tc.tile_pool(name="psumw", bufs=4, space="PSUM") as psw,
        tc.tile_pool(name="dram", bufs=1, space="DRAM") as dp,
    ):
        # Dangling 1-byte AllGather: marks the NEFF as collective so the
        # runtime gang-launches the 8 cores; nothing waits on it.
        cc_in = dp.tile([1, 1], F32)
        cc_out = dp.tile([R, 1], F32)
        nc.gpsimd.collective_compute(
            "AllGather", ALU.bypass, replica_groups=[list(range(R))],
            ins=[cc_in.opt()], outs=[cc_out.opt()],
        )

        x_all = pp.tile([P, T * D], F32)      # row tile i at [:, i*D:(i+1)*D]
        xb_all = pp.tile([P, T * D], BF16)    # bf16 copy of x
        g_all = pp.tile([P, T * D], BF16)     # f * x (bf16)
        xT_all = pp.tile([P, 2 * NL], BF16)   # x.T chunk c at [:, c*NL + i*P]
        W_sb = pp.tile([P, 2 * D], F32)       # W k-chunk kc at [:, kc*D]
        Wb_sb = pp.tile([P, 2 * D], BF16)
        zT_sb = pp.tile([P, 2 * D], BF16)

        ss = pp.tile([P, T], F32)
        invn = pp.tile([P, T], F32)
        nrm = pp.tile([P, T], F32)
        stl = pp.tile([P, T], F32)
        s_t = pp.tile([P, T], F32)
        sq_s = pp.tile([P, T], F32)
        dd = pp.tile([P, T], F32)
        f_t = pp.tile([P, T], F32)

        ident = pp.tile([P, P], F32)
        masks.make_identity(nc, ident[:])
        ones2 = pp.tile([2, P], F32)
        nc.gpsimd.memset(ones2[:], 1.0)

        # Exchange buffers.  t_col / zw_loc accumulate in place; thr/zhr
        # receive the partner's running sum each hop.
        t_col = pp.tile([P, TSLOT], F32)      # my colsum partial, cols 0-1
        thr = [pp.tile([P, TSLOT], F32, name=f"thr{k}") for k in range(3)]
        zw_loc = pp.tile([P, ZSLOT], BF16)    # my (z @ W) partial
        zhr = [pp.tile([P, ZSLOT], BF16, name=f"zhr{k}") for k in range(3)]
        t_row = pp.tile([1, 2 * P], F32)
        tb_sb = pp.tile([P, D], F32)

        nc.gpsimd.memset(t_col[:], 0.0)

        # First t-hop send descriptor on queue 0 (later hops' preps are
        # emitted after the previous hop's trigger so each trigger fires
        # exactly one prep).
        nc.gpsimd.remote_dma_broadcast(
            thr[0][:], t_col[:], rsem_t[0], lsem[0], rdests=_rdests(HOPS[0]),
        )

        for kc in range(2):
            nc.sync.dma_start(W_sb[:, kc * D:(kc + 1) * D], W[kc * P:(kc + 1) * P, :])
        nc.vector.tensor_copy(Wb_sb[:], W_sb[:])

        # ---- phase A: load shard, row norms, colsum(xn) partial ----
        for i in range(T):
            xs = x_all[:, i * D:(i + 1) * D]
            nc.sync.dma_start(xs, x[i * P:(i + 1) * P, :])
            scr = wp.tile([P, D], F32, tag="scr", name=f"scr{i}")
            nc.scalar.activation(scr[:], xs, AF.Square, accum_out=ss[:, i:i + 1])
            nc.vector.tensor_copy(xb_all[:, i * D:(i + 1) * D], xs)
        nc.scalar.activation(nrm[:], ss[:], AF.Sqrt)
        nc.vector.reciprocal(invn[:], nrm[:])

        # t partial in column layout: t_col[p, c] = sum_i (x_i chunk c)^T invn_i
        psum_tc = psw.tile([P, D], F32, tag="pw", name="psum_tc")
        for c in range(2):
            for i in range(T):
                nc.tensor.matmul(
                    psum_tc[:, c:c + 1],
                    lhsT=x_all[:, i * D + c * P:i * D + (c + 1) * P],
                    rhs=invn[:, i:i + 1],
                    start=(i == 0), stop=(i == T - 1),
                )
        nc.vector.tensor_copy(t_col[:, 0:2], psum_tc[:, 0:2])

        # x.T via PE transposes (independent work that overlaps the exchange)
        for i in range(T):
            for c in range(2):
                pt = psw.tile([P, P], F32, tag="pw", name=f"pt{i}_{c}")
                nc.tensor.transpose(
                    pt[:], x_all[:, i * D + c * P: i * D + (c + 1) * P], ident[:]
                )
                nc.vector.tensor_copy(xT_all[:, c * NL + i * P: c * NL + (i + 1) * P], pt[:])

        # ---- t recursive doubling.  No entry barrier is needed:
        # target_bir_lowering is off so there is no per-kernel sem clear —
        # sems are zeroed at NEFF load and remote increments persist even if
        # a peer has not started executing yet.  Each trigger's
        # signals_writable gives it a WAW edge after the t_col producer (the
        # preps predate the producer, so the deferred-RAW edge never forms),
        # orders the hop's consumer add after it, and pins the next hop's
        # prep behind it in the queue-0 FIFO.
        add_t = []
        for k in range(3):
            sig = [t_col[:], thr[k][:]]
            if k < 2:
                # Pins the next hop's prep (emitted below, writing thr[k+1])
                # behind this trigger in the queue-0 FIFO via a WAW edge.
                sig.append(thr[k + 1][:])
            nc.gpsimd.trigger_dma(count=None, signals_writable=tuple(sig))
            if k < 2:
                nc.gpsimd.remote_dma_broadcast(
                    thr[k + 1][:], t_col[:], rsem_t[k + 1], lsem[0],
                    rdests=_rdests(HOPS[k + 1]),
                )
            add_t.append(nc.vector.tensor_add(t_col[:], t_col[:], thr[k][:]))

        # zw-hop send descriptors, queues 1-3 (their own queues: queue-0
        # FIFO order with the t hops is not schedule-stable).
        for k, dq in enumerate(HOPS):
            nc.gpsimd.remote_dma_broadcast(
                zhr[k][:], zw_loc[:], rsem_z[k], lsem[1 + k],
                rdests=_rdests(dq), queue_num=1 + k,
            )

        # ---- phase B: degrees, f, g = f*x, zT partial, zw partial ----
        # t (column layout) -> per-chunk row at partition 0 -> broadcast to
        # 128 partitions via K=1 ones-matmuls (everything base-partition 0).
        psum_tb = psw.tile([P, D], F32, tag="pw", name="psum_tb")
        for c in range(2):
            ptr = psw.tile([P, D], F32, tag="pw", name=f"ptr{c}")
            nc.tensor.transpose(ptr[0:1, 0:P], t_col[:, c:c + 1], ident[:])
            nc.vector.tensor_copy(t_row[0:1, c * P:(c + 1) * P], ptr[0:1, 0:P])
            nc.tensor.matmul(
                psum_tb[:, c * P:(c + 1) * P],
                lhsT=ones2[0:1, :],
                rhs=t_row[0:1, c * P:(c + 1) * P],
                start=True, stop=True,
            )
        nc.vector.tensor_copy(tb_sb[:], psum_tb[:])

        big_scr = pp.tile([P, T * D], F32)
        t_ap = tb_sb[:]
        from concourse.bass_types import AP as _AP
        t_rep = _AP(t_ap.tensor, t_ap.offset, [t_ap.ap[0], [0, T], t_ap.ap[1]])
        x3 = x_all[:].rearrange("p (t d) -> p t d", t=T)
        s3 = big_scr[:].rearrange("p (t d) -> p t d", t=T)
        nc.vector.tensor_mul(s3, x3, t_rep)
        nc.vector.tensor_reduce(stl[:], s3, axis=mybir.AxisListType.X, op=ALU.add)
        nc.vector.tensor_mul(s_t[:], stl[:], invn[:])       # s = rowsum * invn
        nc.scalar.activation(sq_s[:], s_t[:], AF.Sqrt)
        nc.vector.reciprocal(dd[:], sq_s[:])                # d = rsqrt(s)
        nc.vector.tensor_mul(f_t[:], dd[:], invn[:])        # f = d * invn
        for i in range(T):
            nc.scalar.mul(g_all[:, i * D:(i + 1) * D], x_all[:, i * D:(i + 1) * D],
                          f_t[:, i:i + 1])

        psum_zT0 = psp.tile([P, D], F32, name="pzT0")
        psum_zT1 = psp.tile([P, D], F32, name="pzT1")
        for i in range(T):
            for c, pz in ((0, psum_zT0), (1, psum_zT1)):
                nc.tensor.matmul(
                    pz[:], lhsT=xb_all[:, i * D + c * P: i * D + (c + 1) * P],
                    rhs=g_all[:, i * D:(i + 1) * D],
                    start=(i == 0), stop=(i == T - 1),
                )
        for c, pz in ((0, psum_zT0), (1, psum_zT1)):
            nc.vector.tensor_copy(zT_sb[:, c * D:(c + 1) * D], pz[:])

        # zw partial = z_p @ W (fold the W GEMM before the exchange)
        for m in range(2):
            pzw = psw.tile([P, D], F32, tag="pw", name=f"pzw{m}")
            for kc in range(2):
                nc.tensor.matmul(
                    pzw[:], lhsT=zT_sb[:, kc * D + m * P: kc * D + (m + 1) * P],
                    rhs=Wb_sb[:, kc * D:(kc + 1) * D],
                    start=(kc == 0), stop=(kc == 1),
                )
            nc.vector.tensor_copy(zw_loc[:, m * D:(m + 1) * D], pzw[:])

        # ---- zw recursive doubling (hops on queues 1-3) ----
        add_z = []
        for k in range(3):
            nc.gpsimd.trigger_dma(count=None, queue_num=1 + k,
                                  signals_writable=(zw_loc[:], zhr[k][:]))
            add_z.append(nc.vector.tensor_add(zw_loc[:], zw_loc[:], zhr[k][:]))

        # ---- phase C: out = f * (x @ zw) ----
        for i in range(T):
            po = psw.tile([P, D], F32, tag="pw", name=f"po{i}")
            for ka in range(2):
                nc.tensor.matmul(
                    po[:], lhsT=xT_all[:, ka * NL + i * P: ka * NL + (i + 1) * P],
                    rhs=zw_loc[:, ka * D:(ka + 1) * D],
                    start=(ka == 0), stop=(ka == 1),
                )
            o_sb = wp.tile([P, D], F32, tag="osb", name=f"osb{i}")
            nc.scalar.mul(o_sb[:], po[:], f_t[:, i:i + 1])
            nc.sync.dma_start(out[i * P:(i + 1) * P, :], o_sb[:])

    return {"add_t": add_t, "add_z": add_z,
            "rsem_t": rsem_t, "rsem_z": rsem_z, "lsem": lsem}


def _build():
    nc = bacc.Bacc("TRN2", target_bir_lowering=False, debug=False, num_devices=R,
                   num_swdge_queues=4)
    x = nc.dram_tensor("x", [NL, D], F32, kind="ExternalInput")
    W = nc.dram_tensor("W", [D, D], F32, kind="ExternalInput")
    out = nc.dram_tensor("out", [NL, D], F32, kind="ExternalOutput")
    with tile.TileContext(nc) as tc:
        h = _program(tc, x.ap() if hasattr(x, "ap") else x, W.ap() if hasattr(W, "ap") else W, out.ap() if hasattr(out, "ap") else out)
    # Attach the cross-core waits after scheduling (the schedule-time
    # single-core sim cannot model peer sem increments, and added waits
    # only delay — they cannot invalidate the schedule).  Each hop's add
    # waits for the partner's payload (+2 on the hop's remote sem) and for
    # this core's own send of the hop to drain (+16 on the queue's local
    # sem) before overwriting the send buffer.  compile() splits
    # multi-wait instructions into event semaphores automatically.
    for k in range(3):
        h["add_t"][k].wait_op(h["rsem_t"][k], 2, "sem-ge", check=False)
        h["add_t"][k].wait_op(h["lsem"][0], 16 * (k + 1), "sem-ge", check=False)
        h["add_z"][k].wait_op(h["rsem_z"][k], 2, "sem-ge", check=False)
        h["add_z"][k].wait_op(h["lsem"][1 + k], 16, "sem-ge", check=False)
    nc.finalize()
    return nc


def _run(inputs, trace=False):
    if "nc" not in _cache:
        _cache["nc"] = _build()
    nc = _cache["nc"]
    x = np.ascontiguousarray(inputs["x"], dtype=np.float32)
    W = np.ascontiguousarray(inputs["W"], dtype=np.float32)
    in_maps = [{"x": x[r * NL:(r + 1) * NL], "W": W} for r in range(R)]
    res = bass_utils.run_bass_kernel_spmd(
        nc, in_maps, core_ids=list(range(R)), trace=trace,
    )
    out = np.concatenate([res.results[r]["out"] for r in range(R)], axis=0)
    return out, res


def kernel(**inputs) -> np.ndarray:
    out, _ = _run(inputs, trace=False)
    return out


# revision 59
# speedup vs baseline: 49.7215x; 1.0167x over previous
"""Distributed Trainium2 kernel for the dense-graph GNN layer.

Math: with xn = x/||x|| (rows), G = xn@xn.T, d = rsqrt(G@1),
out = (diag(d) G diag(d) x) W.  The N x N Gram matrix is never needed:
  G @ 1        = xn @ t,            t = colsum(xn)            [D]
  diag(d) G diag(d) x = f * (x @ z),  z = x.T @ diag(f) @ x   [D, D]
  f_i = d_i / ||x_i||   (combines both scalings; z is symmetric)
  out = f * (x @ (z @ W))
Each core processes its 1024-row shard; the only cross-core traffic is
(1) a reduction of the [D] colsum partial and (2) a reduction of the
[D, D] (z @ W) partial.  Both reductions run as recursive-doubling
exchanges over direct peer-to-peer SBUF remote DMAs (XOR partners 1, 2,
4), which avoids both the ~70us ncfw collective bringup and the
descriptor flood of a full mesh.  A dangling 1-byte AllGather marks the
NEFF as collective so the runtime gang-launches the 8 cores (without
it, dispatch is staggered by milliseconds).
"""

import os
import sys

import numpy as np

for _p in ("/opt/trn_rl_repo", "/root/.axon_site/_ro/trn_rl_repo"):
    if os.path.isdir(_p) and _p not in sys.path:
        sys.path.insert(0, _p)

import concourse.bacc as bacc
import concourse.mybir as mybir
import concourse.tile as tile
import concourse.masks as masks
from concourse import bass_utils

R = 8                 # cores
N, D = 8192, 256
NL = N // R           # 1024 rows per core
P = 128
T = NL // P           # 8 row tiles per core
F32 = mybir.dt.float32
BF16 = mybir.dt.bfloat16
AF = mybir.ActivationFunctionType
ALU = mybir.AluOpType

TSLOT = 8             # t exchange payload width (f32 cols) = 32 B/partition
ZSLOT = 2 * D         # zw exchange payload width (bf16 cols) = 1 KB/partition
HOPS = (1, 2, 4)      # recursive-doubling XOR distances

_cache = {}


def _rdests(dq):
    """8-slot dest list.  Same-die hops (dq<4) repeat the dest in every
    slot: all 16 lanes carry real (replicated) payload, so no dummy
    descriptors exist — dummy lanes trickle 4-byte descriptors for ~6us
    and would pace the hop.  The cross-die hop (dq=4) may only use the
    D2D-capable slots 4-7 (8 lanes, 4 copies); its 8 dummy lanes drain in
    the background and gate nothing (the accumulation is double-buffered).
    Receiver sem increments: +16 same-die, +8 cross-die."""
    if dq < 4:
        return [(0, dq)] * 8
    return [None, None, None, None, (0, dq), (0, dq), (0, dq), (0, dq)]


RINC = {1: 16, 2: 16, 4: 8}   # per-arrival remote-sem increment by distance


def _program(tc, x, W, out):
    nc = tc.nc
    # Per-hop arrival sems (a shared counter would be ambiguous: a fast
    # partner's hop-2 arrival must not satisfy a hop-1 wait).
    rsem_t = [nc.alloc_semaphore(f"rsem_t{k}") for k in range(3)]
    rsem_z = [nc.alloc_semaphore(f"rsem_z{k}") for k in range(3)]
    # Local (send-drained) sems, one per SWDGE queue.
    lsem = [nc.alloc_semaphore(f"lsem_q{q}") for q in range(4)]
    with (
        tc.tile_pool(name="persist", bufs=1) as pp,
        tc.tile_pool(name="work", bufs=3) as wp,
        tc.tile_pool(name="psum", bufs=1, space="PSUM") as psp,
        tc.tile_pool(name="psumw", bufs=4, space="PSUM") as psw,
        tc.tile_pool(name="dram", bufs=1, space="DRAM") as dp,
    ):
        # Dangling 1-byte AllGather: marks the NEFF as collective so the
        # runtime gang-launches the 8 cores; nothing waits on it.
        cc_in = dp.tile([1, 1], F32)
        cc_out = dp.tile([R, 1], F32)
        nc.gpsimd.collective_compute(
            "AllGather", ALU.bypass, replica_groups=[list(range(R))],
            ins=[cc_in.opt()], outs=[cc_out.opt()],
        )

        x_all = pp.tile([P, T * D], F32)      # row tile i at [:, i*D:(i+1)*D]
        xb_all = pp.tile([P, T * D], BF16)    # bf16 copy of x
        g_all = pp.tile([P, T * D], BF16)     # f * x (bf16)
        xT_all = pp.tile([P, 2 * NL], BF16)   # x.T chunk c at [:, c*NL + i*P]
        W_sb = pp.tile([P, 2 * D], F32)       # W k-chunk kc at [:, kc*D]
        Wb_sb = pp.tile([P, 2 * D], BF16)
        zT_sb = pp.tile([P, 2 * D], BF16)

        ss = pp.tile([P, T], F32)
        invn = pp.tile([P, T], F32)
        nrm = pp.tile([P, T], F32)
        stl = pp.tile([P, T], F32)
        s_t = pp.tile([P, T], F32)
        sq_s = pp.tile([P, T], F32)
        dd = pp.tile([P, T], F32)
        f_t = pp.tile([P, T], F32)

        ident = pp.tile([P, P], F32)
        masks.make_identity(nc, ident[:])
        ones2 = pp.tile([2, P], F32)
        nc.gpsimd.memset(ones2[:], 1.0)

        # Exchange buffers.  t_col / zw_loc accumulate in place; thr/zhr
        # receive the partner's running sum each hop.
        t_col = pp.tile([P, TSLOT], F32)      # my colsum partial, cols 0-1
        thr = [pp.tile([P, TSLOT], F32, name=f"thr{k}") for k in range(3)]
        ta = [t_col] + [pp.tile([P, TSLOT], F32, name=f"ta{k}") for k in range(3)]
        zw_loc = pp.tile([P, ZSLOT], BF16)    # my (z @ W) partial
        zhr = [pp.tile([P, ZSLOT], BF16, name=f"zhr{k}") for k in range(3)]
        za = [zw_loc] + [pp.tile([P, ZSLOT], BF16, name=f"za{k}") for k in range(3)]
        t_row = pp.tile([1, 2 * P], F32)
        tb_sb = pp.tile([P, D], F32)

        nc.gpsimd.memset(t_col[:], 0.0)

        # First t-hop send descriptor on queue 0 (later hops' preps are
        # emitted after the previous hop's trigger so each trigger fires
        # exactly one prep).
        nc.gpsimd.remote_dma_broadcast(
            thr[0][:], t_col[:], rsem_t[0], lsem[0], rdests=_rdests(HOPS[0]),
        )

        for kc in range(2):
            nc.sync.dma_start(W_sb[:, kc * D:(kc + 1) * D], W[kc * P:(kc + 1) * P, :])
        nc.vector.tensor_copy(Wb_sb[:], W_sb[:])

        # ---- phase A: load shard, row norms, colsum(xn) partial ----
        # One 3-D DMA for the whole 1 MB shard (row tile i of the shard is
        # column block i of x_all): one dispatch instead of eight.
        from concourse.bass_types import AP as _AP
        x_src = _AP(x.tensor, x.offset, [[D, P], [P * D, T], [1, D]])
        nc.sync.dma_start(x_all[:].rearrange("p (t d) -> p t d", t=T), x_src)
        for i in range(T):
            xs = x_all[:, i * D:(i + 1) * D]
            scr = wp.tile([P, D], F32, tag="scr", name=f"scr{i}")
            nc.scalar.activation(scr[:], xs, AF.Square, accum_out=ss[:, i:i + 1])
        nc.vector.tensor_copy(xb_all[:], x_all[:])
        nc.scalar.activation(nrm[:], ss[:], AF.Sqrt)
        nc.vector.reciprocal(invn[:], nrm[:])
        invn_b = pp.tile([P, T], BF16)
        nc.vector.tensor_copy(invn_b[:], invn[:])

        # t partial in column layout: t_col[p, c] = sum_i (x_i chunk c)^T invn_i
        # (bf16 operands: one-pass matmuls, ~3x faster than fp32)
        psum_tc = psw.tile([P, D], F32, tag="pw", name="psum_tc")
        for c in range(2):
            for i in range(T):
                nc.tensor.matmul(
                    psum_tc[:, c:c + 1],
                    lhsT=xb_all[:, i * D + c * P:i * D + (c + 1) * P],
                    rhs=invn_b[:, i:i + 1],
                    start=(i == 0), stop=(i == T - 1),
                )
        nc.vector.tensor_copy(t_col[:, 0:2], psum_tc[:, 0:2])

        # x.T via PE transposes (independent work that overlaps the exchange)
        for i in range(T):
            for c in range(2):
                pt = psw.tile([P, P], F32, tag="pw", name=f"pt{i}_{c}")
                nc.tensor.transpose(
                    pt[:], x_all[:, i * D + c * P: i * D + (c + 1) * P], ident[:]
                )
                nc.vector.tensor_copy(xT_all[:, c * NL + i * P: c * NL + (i + 1) * P], pt[:])

        # ---- t recursive doubling.  No entry barrier is needed:
        # target_bir_lowering is off so there is no per-kernel sem clear —
        # sems are zeroed at NEFF load and remote increments persist even if
        # a peer has not started executing yet.  Each trigger's
        # signals_writable gives it a WAW edge after the t_col producer (the
        # preps predate the producer, so the deferred-RAW edge never forms),
        # orders the hop's consumer add after it, and pins the next hop's
        # prep behind it in the queue-0 FIFO.
        add_t = []
        for k in range(3):
            sig = [ta[k][:], thr[k][:]]
            if k < 2:
                # Pins the next hop's prep (emitted below, writing thr[k+1])
                # behind this trigger in the queue-0 FIFO via a WAW edge.
                sig.append(thr[k + 1][:])
            nc.gpsimd.trigger_dma(count=None, signals_writable=tuple(sig))
            if k < 2:
                nc.gpsimd.remote_dma_broadcast(
                    thr[k + 1][:], ta[k + 1][:], rsem_t[k + 1], lsem[0],
                    rdests=_rdests(HOPS[k + 1]),
                )
            # Double-buffered: the sum lands in a fresh tile, so the hop's
            # in-flight send never races the accumulation (no local-sem wait).
            add_t.append(nc.vector.tensor_add(ta[k + 1][:], ta[k][:], thr[k][:]))

        # zw-hop send descriptors, queues 1-3 (their own queues: queue-0
        # FIFO order with the t hops is not schedule-stable).
        for k, dq in enumerate(HOPS):
            nc.gpsimd.remote_dma_broadcast(
                zhr[k][:], za[k][:], rsem_z[k], lsem[1 + k],
                rdests=_rdests(dq), queue_num=1 + k,
            )

        # ---- phase B: degrees, f, g = f*x, zT partial, zw partial ----
        # t (column layout) -> per-chunk row at partition 0 -> broadcast to
        # 128 partitions via K=1 ones-matmuls (everything base-partition 0).
        psum_tb = psw.tile([P, D], F32, tag="pw", name="psum_tb")
        for c in range(2):
            ptr = psw.tile([P, D], F32, tag="pw", name=f"ptr{c}")
            nc.tensor.transpose(ptr[0:1, 0:P], ta[3][:, c:c + 1], ident[:])
            nc.vector.tensor_copy(t_row[0:1, c * P:(c + 1) * P], ptr[0:1, 0:P])
            nc.tensor.matmul(
                psum_tb[:, c * P:(c + 1) * P],
                lhsT=ones2[0:1, :],
                rhs=t_row[0:1, c * P:(c + 1) * P],
                start=True, stop=True,
            )
        nc.vector.tensor_copy(tb_sb[:], psum_tb[:])

        big_scr = pp.tile([P, T * D], BF16)
        t_ap = tb_sb[:]
        t_rep = _AP(t_ap.tensor, t_ap.offset, [t_ap.ap[0], [0, T], t_ap.ap[1]])
        x3 = x_all[:].rearrange("p (t d) -> p t d", t=T)
        s3 = big_scr[:].rearrange("p (t d) -> p t d", t=T)
        nc.vector.tensor_mul(s3, x3, t_rep)
        nc.vector.tensor_reduce(stl[:], s3, axis=mybir.AxisListType.X, op=ALU.add)
        nc.vector.tensor_mul(s_t[:], stl[:], invn[:])       # s = rowsum * invn
        nc.scalar.activation(sq_s[:], s_t[:], AF.Sqrt)
        nc.vector.reciprocal(dd[:], sq_s[:])                # d = rsqrt(s)
        nc.vector.tensor_mul(f_t[:], dd[:], invn[:])        # f = d * invn
        for i in range(T):
            nc.scalar.mul(g_all[:, i * D:(i + 1) * D], x_all[:, i * D:(i + 1) * D],
                          f_t[:, i:i + 1])

        psum_zT0 = psp.tile([P, D], F32, name="pzT0")
        psum_zT1 = psp.tile([P, D], F32, name="pzT1")
        for i in range(T):
            for c, pz in ((0, psum_zT0), (1, psum_zT1)):
                nc.tensor.matmul(
                    pz[:], lhsT=xb_all[:, i * D + c * P: i * D + (c + 1) * P],
                    rhs=g_all[:, i * D:(i + 1) * D],
                    start=(i == 0), stop=(i == T - 1),
                )
        for c, pz in ((0, psum_zT0), (1, psum_zT1)):
            nc.vector.tensor_copy(zT_sb[:, c * D:(c + 1) * D], pz[:])

        # zw partial = z_p @ W (fold the W GEMM before the exchange)
        for m in range(2):
            pzw = psw.tile([P, D], F32, tag="pw", name=f"pzw{m}")
            for kc in range(2):
                nc.tensor.matmul(
                    pzw[:], lhsT=zT_sb[:, kc * D + m * P: kc * D + (m + 1) * P],
                    rhs=Wb_sb[:, kc * D:(kc + 1) * D],
                    start=(kc == 0), stop=(kc == 1),
                )
            nc.vector.tensor_copy(zw_loc[:, m * D:(m + 1) * D], pzw[:])

        # ---- zw recursive doubling (hops on queues 1-3) ----
        add_z = []
        for k in range(3):
            nc.gpsimd.trigger_dma(count=None, queue_num=1 + k,
                                  signals_writable=(za[k][:], zhr[k][:]))
            add_z.append(nc.vector.tensor_add(za[k + 1][:], za[k][:], zhr[k][:]))

        # ---- phase C: out = f * (x @ zw) ----
        for i in range(T):
            po = psw.tile([P, D], F32, tag="pw", name=f"po{i}")
            for ka in range(2):
                nc.tensor.matmul(
                    po[:], lhsT=xT_all[:, ka * NL + i * P: ka * NL + (i + 1) * P],
                    rhs=za[3][:, ka * D:(ka + 1) * D],
                    start=(ka == 0), stop=(ka == 1),
                )
            o_sb = wp.tile([P, D], F32, tag="osb", name=f"osb{i}")
            nc.scalar.mul(o_sb[:], po[:], f_t[:, i:i + 1])
            nc.sync.dma_start(out[i * P:(i + 1) * P, :], o_sb[:])

    return {"add_t": add_t, "add_z": add_z,
            "rsem_t": rsem_t, "rsem_z": rsem_z, "lsem": lsem}


def _build():
    nc = bacc.Bacc("TRN2", target_bir_lowering=False, debug=False, num_devices=R,
                   num_swdge_queues=4)
    x = nc.dram_tensor("x", [NL, D], F32, kind="ExternalInput")
    W = nc.dram_tensor("W", [D, D], F32, kind="ExternalInput")
    out = nc.dram_tensor("out", [NL, D], F32, kind="ExternalOutput")
    with tile.TileContext(nc) as tc:
        h = _program(tc, x.ap() if hasattr(x, "ap") else x, W.ap() if hasattr(W, "ap") else W, out.ap() if hasattr(out, "ap") else out)
    # Attach the cross-core waits after scheduling (the schedule-time
    # single-core sim cannot model peer sem increments, and added waits
    # only delay — they cannot invalidate the schedule).  Each hop's add
    # waits for the partner's payload (+2 on the hop's remote sem) and for
    # this core's own send of the hop to drain (+16 on the queue's local
    # sem) before overwriting the send buffer.  compile() splits
    # multi-wait instructions into event semaphores automatically.
    for k in range(3):
        h["add_t"][k].wait_op(h["rsem_t"][k], RINC[HOPS[k]], "sem-ge", check=False)
        h["add_z"][k].wait_op(h["rsem_z"][k], RINC[HOPS[k]], "sem-ge", check=False)
    nc.finalize()
    return nc


def _run(inputs, trace=False):
    if "nc" not in _cache:
        _cache["nc"] = _build()
    nc = _cache["nc"]
    x = np.ascontiguousarray(inputs["x"], dtype=np.float32)
    W = np.ascontiguousarray(inputs["W"], dtype=np.float32)
    in_maps = [{"x": x[r * NL:(r + 1) * NL], "W": W} for r in range(R)]
    res = bass_utils.run_bass_kernel_spmd(
        nc, in_maps, core_ids=list(range(R)), trace=trace,
    )
    out = np.concatenate([res.results[r]["out"] for r in range(R)], axis=0)
    return out, res


def kernel(**inputs) -> np.ndarray:
    out, _ = _run(inputs, trace=False)
    return out


# revision 62
# speedup vs baseline: 55.3142x; 1.1125x over previous
"""Distributed Trainium2 kernel for the dense-graph GNN layer.

Math: with xn = x/||x|| (rows), G = xn@xn.T, d = rsqrt(G@1),
out = (diag(d) G diag(d) x) W.  The N x N Gram matrix is never needed:
  G @ 1        = xn @ t,            t = colsum(xn)            [D]
  diag(d) G diag(d) x = f * (x @ z),  z = x.T @ diag(f) @ x   [D, D]
  f_i = d_i / ||x_i||   (combines both scalings; z is symmetric)
  out = f * (x @ (z @ W))
Each core processes its 1024-row shard; the only cross-core traffic is
(1) a reduction of the [D] colsum partial and (2) a reduction of the
[D, D] (z @ W) partial.  Both reductions run as recursive-doubling
exchanges over direct peer-to-peer SBUF remote DMAs (XOR partners 1, 2,
4), which avoids both the ~70us ncfw collective bringup and the
descriptor flood of a full mesh.  A dangling 1-byte AllGather marks the
NEFF as collective so the runtime gang-launches the 8 cores (without
it, dispatch is staggered by milliseconds).
"""

import os
import sys

import numpy as np

for _p in ("/opt/trn_rl_repo", "/root/.axon_site/_ro/trn_rl_repo"):
    if os.path.isdir(_p) and _p not in sys.path:
        sys.path.insert(0, _p)

import concourse.bacc as bacc
import concourse.mybir as mybir
import concourse.tile as tile
import concourse.masks as masks
from concourse import bass_utils

R = 8                 # cores
N, D = 8192, 256
NL = N // R           # 1024 rows per core
P = 128
T = NL // P           # 8 row tiles per core
F32 = mybir.dt.float32
BF16 = mybir.dt.bfloat16
AF = mybir.ActivationFunctionType
ALU = mybir.AluOpType

TSLOT = 8             # t exchange payload width (f32 cols) = 32 B/partition
ZSLOT = 2 * D         # zw exchange payload width (bf16 cols) = 1 KB/partition
HOPS = (4, 2, 1)      # recursive-doubling XOR distances (cross-die first)

_cache = {}


def _rdests(dq):
    """8-slot dest list.  dq=1 repeats the dest in every slot (all 16
    lanes real, no dummy descriptors — it is the last hop, and dummy
    lanes trickle 4-byte descriptors for ~6us which would delay the final
    queue drain).  dq=2 uses 4 same-die slots (half the wire; its dummy
    lanes drain in the background on their own queue and gate nothing —
    the accumulation is double-buffered).  dq=4 may only use the
    D2D-capable slots 4-7.  Receiver sem increments: 2 per real slot."""
    if dq == 1:
        return [(0, dq)] * 8
    if dq == 2:
        return [(0, dq)] * 4 + [None] * 4
    return [None, None, None, None, (0, dq), (0, dq), (0, dq), (0, dq)]


RINC = {1: 16, 2: 8, 4: 8}    # per-arrival remote-sem increment by distance


def _program(tc, x, W, out):
    nc = tc.nc
    # Per-hop arrival sems (a shared counter would be ambiguous: a fast
    # partner's hop-2 arrival must not satisfy a hop-1 wait).
    rsem_t = [nc.alloc_semaphore(f"rsem_t{k}") for k in range(3)]
    rsem_z = [nc.alloc_semaphore(f"rsem_z{k}") for k in range(3)]
    # Local (send-drained) sems, one per SWDGE queue.
    lsem = [nc.alloc_semaphore(f"lsem_q{q}") for q in range(4)]
    with (
        tc.tile_pool(name="persist", bufs=1) as pp,
        tc.tile_pool(name="work", bufs=3) as wp,
        tc.tile_pool(name="psum", bufs=1, space="PSUM") as psp,
        tc.tile_pool(name="psumw", bufs=4, space="PSUM") as psw,
        tc.tile_pool(name="dram", bufs=1, space="DRAM") as dp,
    ):
        # Dangling 1-byte AllGather: marks the NEFF as collective so the
        # runtime gang-launches the 8 cores; nothing waits on it.
        cc_in = dp.tile([1, 1], F32)
        cc_out = dp.tile([R, 1], F32)
        nc.gpsimd.collective_compute(
            "AllGather", ALU.bypass, replica_groups=[list(range(R))],
            ins=[cc_in.opt()], outs=[cc_out.opt()],
        )

        x_all = pp.tile([P, T * D], F32)      # row tile i at [:, i*D:(i+1)*D]
        xb_all = pp.tile([P, T * D], BF16)    # bf16 copy of x
        g_all = pp.tile([P, T * D], BF16)     # f * x (bf16)
        xT_all = pp.tile([P, 2 * NL], BF16)   # x.T chunk c at [:, c*NL + i*P]
        W_sb = pp.tile([P, 2 * D], F32)       # W k-chunk kc at [:, kc*D]
        Wb_sb = pp.tile([P, 2 * D], BF16)
        zT_sb = pp.tile([P, 2 * D], BF16)

        ss = pp.tile([P, T], F32)
        invn = pp.tile([P, T], F32)
        nrm = pp.tile([P, T], F32)
        stl = pp.tile([P, T], F32)
        s_t = pp.tile([P, T], F32)
        sq_s = pp.tile([P, T], F32)
        dd = pp.tile([P, T], F32)
        f_t = pp.tile([P, T], F32)

        ident = pp.tile([P, P], F32)
        masks.make_identity(nc, ident[:])

        # Exchange buffers.  t_col / zw_loc accumulate in place; thr/zhr
        # receive the partner's running sum each hop.
        t_col = pp.tile([P, TSLOT], F32)      # my colsum partial, cols 0-1
        thr = [pp.tile([P, TSLOT], F32, name=f"thr{k}") for k in range(3)]
        ta = [t_col] + [pp.tile([P, TSLOT], F32, name=f"ta{k}") for k in range(3)]
        zw_loc = pp.tile([P, ZSLOT], BF16)    # my (z @ W) partial
        zhr = [pp.tile([P, ZSLOT], BF16, name=f"zhr{k}") for k in range(3)]
        za = [zw_loc] + [pp.tile([P, ZSLOT], BF16, name=f"za{k}") for k in range(3)]

        nc.gpsimd.memset(t_col[:], 0.0)

        # Hop k of both reductions lives on SWDGE queue k: a queue never
        # hosts two consecutive hops, so one hop's background dummy-lane
        # trickle cannot delay the next hop's descriptors.  The t-hop
        # preps are emitted here (desc-gen only, reads deferred).
        for k in range(3):
            nc.gpsimd.remote_dma_broadcast(
                thr[k][:], ta[k][:], rsem_t[k], lsem[k],
                rdests=_rdests(HOPS[k]), queue_num=k,
            )

        for kc in range(2):
            nc.sync.dma_start(W_sb[:, kc * D:(kc + 1) * D], W[kc * P:(kc + 1) * P, :])
        nc.vector.tensor_copy(Wb_sb[:], W_sb[:])

        # ---- phase A: load shard, row norms, colsum(xn) partial ----
        # One 3-D DMA for the whole 1 MB shard (row tile i of the shard is
        # column block i of x_all): one dispatch instead of eight.
        from concourse.bass_types import AP as _AP
        x_src = _AP(x.tensor, x.offset, [[D, P], [P * D, T], [1, D]])
        nc.sync.dma_start(x_all[:].rearrange("p (t d) -> p t d", t=T), x_src)
        for i in range(T):
            xs = x_all[:, i * D:(i + 1) * D]
            scr = wp.tile([P, D], F32, tag="scr", name=f"scr{i}")
            nc.scalar.activation(scr[:], xs, AF.Square, accum_out=ss[:, i:i + 1])
        nc.vector.tensor_copy(xb_all[:], x_all[:])
        nc.scalar.activation(nrm[:], ss[:], AF.Sqrt)
        nc.vector.reciprocal(invn[:], nrm[:])
        invn_b = pp.tile([P, T], BF16)
        nc.vector.tensor_copy(invn_b[:], invn[:])

        # t partial in column layout: t_col[p, c] = sum_i (x_i chunk c)^T invn_i
        # (bf16 operands: one-pass matmuls, ~3x faster than fp32)
        psum_tc = psw.tile([P, D], F32, tag="pw", name="psum_tc")
        for c in range(2):
            for i in range(T):
                nc.tensor.matmul(
                    psum_tc[:, c:c + 1],
                    lhsT=xb_all[:, i * D + c * P:i * D + (c + 1) * P],
                    rhs=invn_b[:, i:i + 1],
                    start=(i == 0), stop=(i == T - 1),
                )
        nc.vector.tensor_copy(t_col[:, 0:2], psum_tc[:, 0:2])

        # x.T via PE transposes (independent work that overlaps the exchange)
        for i in range(T):
            for c in range(2):
                pt = psw.tile([P, P], F32, tag="pw", name=f"pt{i}_{c}")
                nc.tensor.transpose(
                    pt[:], x_all[:, i * D + c * P: i * D + (c + 1) * P], ident[:]
                )
                nc.vector.tensor_copy(xT_all[:, c * NL + i * P: c * NL + (i + 1) * P], pt[:])

        # ---- t recursive doubling.  No entry barrier is needed:
        # target_bir_lowering is off so there is no per-kernel sem clear —
        # sems are zeroed at NEFF load and remote increments persist even if
        # a peer has not started executing yet.  Each trigger's
        # signals_writable gives it a WAW edge after the t_col producer (the
        # preps predate the producer, so the deferred-RAW edge never forms),
        # orders the hop's consumer add after it, and pins the next hop's
        # prep behind it in the queue-0 FIFO.
        add_t = []
        for k in range(3):
            # signals: WAW after the ta[k] producer, before the thr[k]
            # consumer, and pinning queue-k's zw prep (writes zhr[k])
            # behind this trigger in the FIFO.
            nc.gpsimd.trigger_dma(
                count=None, queue_num=k,
                signals_writable=(ta[k][:], thr[k][:], zhr[k][:]),
            )
            # Double-buffered: the sum lands in a fresh tile, so the hop's
            # in-flight send never races the accumulation (no local-sem wait).
            add_t.append(nc.vector.tensor_add(ta[k + 1][:], ta[k][:], thr[k][:]))

        # zw-hop send descriptors, queue k (pinned behind t-trigger k).
        for k, dq in enumerate(HOPS):
            nc.gpsimd.remote_dma_broadcast(
                zhr[k][:], za[k][:], rsem_z[k], lsem[k],
                rdests=_rdests(dq), queue_num=k,
            )

        # ---- phase B: degrees, f, g = f*x, zT partial, zw partial ----
        # Per-row dot products s = x . t directly on the PE using the
        # already-transposed x: psum_s[:, i] = sum_c xT(c,i)^T @ t_col_b[c]
        # — no 128-partition broadcast of t and no big multiply+reduce.
        tcol_b = pp.tile([P, 2], BF16)
        nc.vector.tensor_copy(tcol_b[:], ta[3][:, 0:2])
        psum_s = psw.tile([P, D], F32, tag="pw", name="psum_s")
        for i in range(T):
            for c in range(2):
                nc.tensor.matmul(
                    psum_s[:, i:i + 1],
                    lhsT=xT_all[:, c * NL + i * P:c * NL + (i + 1) * P],
                    rhs=tcol_b[:, c:c + 1],
                    start=(c == 0), stop=(c == 1),
                )
        nc.vector.tensor_copy(stl[:], psum_s[:, 0:T])
        nc.vector.tensor_mul(s_t[:], stl[:], invn[:])       # s = rowsum * invn
        nc.scalar.activation(sq_s[:], s_t[:], AF.Sqrt)
        nc.vector.reciprocal(dd[:], sq_s[:])                # d = rsqrt(s)
        nc.vector.tensor_mul(f_t[:], dd[:], invn[:])        # f = d * invn
        for i in range(T):
            nc.scalar.mul(g_all[:, i * D:(i + 1) * D], x_all[:, i * D:(i + 1) * D],
                          f_t[:, i:i + 1])

        psum_zT0 = psp.tile([P, D], F32, name="pzT0")
        psum_zT1 = psp.tile([P, D], F32, name="pzT1")
        for i in range(T):
            for c, pz in ((0, psum_zT0), (1, psum_zT1)):
                nc.tensor.matmul(
                    pz[:], lhsT=xb_all[:, i * D + c * P: i * D + (c + 1) * P],
                    rhs=g_all[:, i * D:(i + 1) * D],
                    start=(i == 0), stop=(i == T - 1),
                )
        for c, pz in ((0, psum_zT0), (1, psum_zT1)):
            nc.vector.tensor_copy(zT_sb[:, c * D:(c + 1) * D], pz[:])

        # zw partial = z_p @ W (fold the W GEMM before the exchange)
        for m in range(2):
            pzw = psw.tile([P, D], F32, tag="pw", name=f"pzw{m}")
            for kc in range(2):
                nc.tensor.matmul(
                    pzw[:], lhsT=zT_sb[:, kc * D + m * P: kc * D + (m + 1) * P],
                    rhs=Wb_sb[:, kc * D:(kc + 1) * D],
                    start=(kc == 0), stop=(kc == 1),
                )
            nc.vector.tensor_copy(zw_loc[:, m * D:(m + 1) * D], pzw[:])

        # ---- zw recursive doubling (hops on queues 1-3) ----
        add_z = []
        for k in range(3):
            nc.gpsimd.trigger_dma(count=None, queue_num=k,
                                  signals_writable=(za[k][:], zhr[k][:]))
            add_z.append(nc.vector.tensor_add(za[k + 1][:], za[k][:], zhr[k][:]))

        # ---- phase C: out = f * (x @ zw) ----
        for i in range(T):
            po = psw.tile([P, D], F32, tag="pw", name=f"po{i}")
            for ka in range(2):
                nc.tensor.matmul(
                    po[:], lhsT=xT_all[:, ka * NL + i * P: ka * NL + (i + 1) * P],
                    rhs=za[3][:, ka * D:(ka + 1) * D],
                    start=(ka == 0), stop=(ka == 1),
                )
            o_sb = wp.tile([P, D], F32, tag="osb", name=f"osb{i}")
            nc.scalar.mul(o_sb[:], po[:], f_t[:, i:i + 1])
            nc.sync.dma_start(out[i * P:(i + 1) * P, :], o_sb[:])

    return {"add_t": add_t, "add_z": add_z,
            "rsem_t": rsem_t, "rsem_z": rsem_z, "lsem": lsem}


def _build():
    nc = bacc.Bacc("TRN2", target_bir_lowering=False, debug=False, num_devices=R,
                   num_swdge_queues=4)
    x = nc.dram_tensor("x", [NL, D], F32, kind="ExternalInput")
    W = nc.dram_tensor("W", [D, D], F32, kind="ExternalInput")
    out = nc.dram_tensor("out", [NL, D], F32, kind="ExternalOutput")
    with tile.TileContext(nc) as tc:
        h = _program(tc, x.ap() if hasattr(x, "ap") else x, W.ap() if hasattr(W, "ap") else W, out.ap() if hasattr(out, "ap") else out)
    # Attach the cross-core waits after scheduling (the schedule-time
    # single-core sim cannot model peer sem increments, and added waits
    # only delay — they cannot invalidate the schedule).  Each hop's add
    # waits for the partner's payload (+2 on the hop's remote sem) and for
    # this core's own send of the hop to drain (+16 on the queue's local
    # sem) before overwriting the send buffer.  compile() splits
    # multi-wait instructions into event semaphores automatically.
    for k in range(3):
        h["add_t"][k].wait_op(h["rsem_t"][k], RINC[HOPS[k]], "sem-ge", check=False)
        h["add_z"][k].wait_op(h["rsem_z"][k], RINC[HOPS[k]], "sem-ge", check=False)
    nc.finalize()
    return nc


def _run(inputs, trace=False):
    if "nc" not in _cache:
        _cache["nc"] = _build()
    nc = _cache["nc"]
    x = np.ascontiguousarray(inputs["x"], dtype=np.float32)
    W = np.ascontiguousarray(inputs["W"], dtype=np.float32)
    in_maps = [{"x": x[r * NL:(r + 1) * NL], "W": W} for r in range(R)]
    res = bass_utils.run_bass_kernel_spmd(
        nc, in_maps, core_ids=list(range(R)), trace=trace,
    )
    out = np.concatenate([res.results[r]["out"] for r in range(R)], axis=0)
    return out, res


def kernel(**inputs) -> np.ndarray:
    out, _ = _run(inputs, trace=False)
    return out


# revision 63
# speedup vs baseline: 68.4285x; 1.2371x over previous
"""Distributed Trainium2 kernel for the dense-graph GNN layer.

Math: with xn = x/||x|| (rows), G = xn@xn.T, d = rsqrt(G@1),
out = (diag(d) G diag(d) x) W.  The N x N Gram matrix is never needed:
  G @ 1        = xn @ t,            t = colsum(xn)            [D]
  diag(d) G diag(d) x = f * (x @ z),  z = x.T @ diag(f) @ x   [D, D]
  f_i = d_i / ||x_i||   (combines both scalings; z is symmetric)
  out = f * (x @ (z @ W))
Each core processes its 1024-row shard; the only cross-core traffic is
(1) a reduction of the [D] colsum partial and (2) a reduction of the
[D, D] (z @ W) partial.  Both reductions run as recursive-doubling
exchanges over direct peer-to-peer SBUF remote DMAs (XOR partners 1, 2,
4), which avoids both the ~70us ncfw collective bringup and the
descriptor flood of a full mesh.  A dangling 1-byte AllGather marks the
NEFF as collective so the runtime gang-launches the 8 cores (without
it, dispatch is staggered by milliseconds).
"""

import os
import sys

import numpy as np

for _p in ("/opt/trn_rl_repo", "/root/.axon_site/_ro/trn_rl_repo"):
    if os.path.isdir(_p) and _p not in sys.path:
        sys.path.insert(0, _p)

import concourse.bacc as bacc
import concourse.mybir as mybir
import concourse.tile as tile
import concourse.masks as masks
from concourse import bass_utils

R = 8                 # cores
N, D = 8192, 256
NL = N // R           # 1024 rows per core
P = 128
T = NL // P           # 8 row tiles per core
F32 = mybir.dt.float32
BF16 = mybir.dt.bfloat16
AF = mybir.ActivationFunctionType
ALU = mybir.AluOpType

TSLOT = 8             # t exchange payload width (f32 cols) = 32 B/partition
ZSLOT = 2 * D         # zw exchange payload width (bf16 cols) = 1 KB/partition
HOPS = (4, 2, 1)      # recursive-doubling XOR distances (cross-die first)

_cache = {}


def _rdests(dq):
    """8-slot dest list.  dq=1 repeats the dest in every slot (all 16
    lanes real, no dummy descriptors — it is the last hop, and dummy
    lanes trickle 4-byte descriptors for ~6us which would delay the final
    queue drain).  dq=2 uses 4 same-die slots (half the wire; its dummy
    lanes drain in the background on their own queue and gate nothing —
    the accumulation is double-buffered).  dq=4 may only use the
    D2D-capable slots 4-7.  Receiver sem increments: 2 per real slot."""
    if dq < 4:
        return [(0, dq)] * 8
    return [None, None, None, None, (0, dq), (0, dq), (0, dq), (0, dq)]


RINC = {1: 16, 2: 16, 4: 8}   # per-arrival remote-sem increment by distance


def _program(tc, x, W, out):
    nc = tc.nc
    # Per-hop arrival sems (a shared counter would be ambiguous: a fast
    # partner's hop-2 arrival must not satisfy a hop-1 wait).
    rsem_t = [nc.alloc_semaphore(f"rsem_t{k}") for k in range(3)]
    rsem_z = [nc.alloc_semaphore(f"rsem_z{k}") for k in range(3)]
    # Local (send-drained) sems, one per SWDGE queue.
    lsem = [nc.alloc_semaphore(f"lsem_q{q}") for q in range(4)]
    with (
        tc.tile_pool(name="persist", bufs=1) as pp,
        tc.tile_pool(name="work", bufs=3) as wp,
        tc.tile_pool(name="psum", bufs=1, space="PSUM") as psp,
        tc.tile_pool(name="psumw", bufs=4, space="PSUM") as psw,
        tc.tile_pool(name="dram", bufs=1, space="DRAM") as dp,
    ):
        # Dangling 1-byte AllGather: marks the NEFF as collective so the
        # runtime gang-launches the 8 cores; nothing waits on it.
        cc_in = dp.tile([1, 1], F32)
        cc_out = dp.tile([R, 1], F32)
        nc.gpsimd.collective_compute(
            "AllGather", ALU.bypass, replica_groups=[list(range(R))],
            ins=[cc_in.opt()], outs=[cc_out.opt()],
        )

        x_all = pp.tile([P, T * D], F32)      # row tile i at [:, i*D:(i+1)*D]
        xb_all = pp.tile([P, T * D], BF16)    # bf16 copy of x
        g_all = pp.tile([P, T * D], BF16)     # f * x (bf16)
        xT_all = pp.tile([P, 2 * NL], BF16)   # x.T chunk c at [:, c*NL + i*P]
        W_sb = pp.tile([P, 2 * D], F32)       # W k-chunk kc at [:, kc*D]
        Wb_sb = pp.tile([P, 2 * D], BF16)
        zT_sb = pp.tile([P, 2 * D], BF16)

        ss = pp.tile([P, T], F32)
        invn = pp.tile([P, T], F32)
        nrm = pp.tile([P, T], F32)
        stl = pp.tile([P, T], F32)
        s_t = pp.tile([P, T], F32)
        sq_s = pp.tile([P, T], F32)
        dd = pp.tile([P, T], F32)
        f_t = pp.tile([P, T], F32)

        ident = pp.tile([P, P], F32)
        masks.make_identity(nc, ident[:])

        # Exchange buffers.  t_col / zw_loc accumulate in place; thr/zhr
        # receive the partner's running sum each hop.
        t_col = pp.tile([P, TSLOT], F32)      # my colsum partial, cols 0-1
        thr = [pp.tile([P, TSLOT], F32, name=f"thr{k}") for k in range(3)]
        ta = [t_col] + [pp.tile([P, TSLOT], F32, name=f"ta{k}") for k in range(3)]
        zw_loc = pp.tile([P, ZSLOT], BF16)    # my (z @ W) partial
        zhr = [pp.tile([P, ZSLOT], BF16, name=f"zhr{k}") for k in range(3)]
        za = [zw_loc] + [pp.tile([P, ZSLOT], BF16, name=f"za{k}") for k in range(3)]

        nc.gpsimd.memset(t_col[:], 0.0)

        # Hop k of both reductions lives on SWDGE queue k: a queue never
        # hosts two consecutive hops, so one hop's background dummy-lane
        # trickle cannot delay the next hop's descriptors.  The t-hop
        # preps are emitted here (desc-gen only, reads deferred).
        for k in range(3):
            nc.gpsimd.remote_dma_broadcast(
                thr[k][:], ta[k][:], rsem_t[k], lsem[k],
                rdests=_rdests(HOPS[k]), queue_num=k,
            )

        for kc in range(2):
            nc.sync.dma_start(W_sb[:, kc * D:(kc + 1) * D], W[kc * P:(kc + 1) * P, :])
        nc.vector.tensor_copy(Wb_sb[:], W_sb[:])

        # ---- phase A: load shard, row norms, colsum(xn) partial ----
        from concourse.bass_types import AP as _AP
        for i in range(T):
            xs = x_all[:, i * D:(i + 1) * D]
            nc.sync.dma_start(xs, x[i * P:(i + 1) * P, :])
            scr = wp.tile([P, D], F32, tag="scr", name=f"scr{i}")
            nc.scalar.activation(scr[:], xs, AF.Square, accum_out=ss[:, i:i + 1])
            nc.vector.tensor_copy(xb_all[:, i * D:(i + 1) * D], xs)
        nc.scalar.activation(nrm[:], ss[:], AF.Sqrt)
        nc.vector.reciprocal(invn[:], nrm[:])
        invn_b = pp.tile([P, T], BF16)
        nc.vector.tensor_copy(invn_b[:], invn[:])

        # t partial in column layout: t_col[p, c] = sum_i (x_i chunk c)^T invn_i
        # (bf16 operands: one-pass matmuls, ~3x faster than fp32)
        psum_tc = psw.tile([P, D], F32, tag="pw", name="psum_tc")
        for c in range(2):
            for i in range(T):
                nc.tensor.matmul(
                    psum_tc[:, c:c + 1],
                    lhsT=xb_all[:, i * D + c * P:i * D + (c + 1) * P],
                    rhs=invn_b[:, i:i + 1],
                    start=(i == 0), stop=(i == T - 1),
                )
        nc.vector.tensor_copy(t_col[:, 0:2], psum_tc[:, 0:2])

        # x.T via PE transposes (independent work that overlaps the exchange)
        for i in range(T):
            for c in range(2):
                pt = psw.tile([P, P], F32, tag="pw", name=f"pt{i}_{c}")
                nc.tensor.transpose(
                    pt[:], x_all[:, i * D + c * P: i * D + (c + 1) * P], ident[:]
                )
                nc.vector.tensor_copy(xT_all[:, c * NL + i * P: c * NL + (i + 1) * P], pt[:])

        # ---- t recursive doubling.  No entry barrier is needed:
        # target_bir_lowering is off so there is no per-kernel sem clear —
        # sems are zeroed at NEFF load and remote increments persist even if
        # a peer has not started executing yet.  Each trigger's
        # signals_writable gives it a WAW edge after the t_col producer (the
        # preps predate the producer, so the deferred-RAW edge never forms),
        # orders the hop's consumer add after it, and pins the next hop's
        # prep behind it in the queue-0 FIFO.
        add_t = []
        for k in range(3):
            # signals: WAW after the ta[k] producer, before the thr[k]
            # consumer, and pinning queue-k's zw prep (writes zhr[k])
            # behind this trigger in the FIFO.
            nc.gpsimd.trigger_dma(
                count=None, queue_num=k,
                signals_writable=(ta[k][:], thr[k][:], zhr[k][:]),
            )
            # Double-buffered: the sum lands in a fresh tile, so the hop's
            # in-flight send never races the accumulation (no local-sem wait).
            add_t.append(nc.vector.tensor_add(ta[k + 1][:], ta[k][:], thr[k][:]))

        # zw-hop send descriptors, queue k (pinned behind t-trigger k).
        for k, dq in enumerate(HOPS):
            nc.gpsimd.remote_dma_broadcast(
                zhr[k][:], za[k][:], rsem_z[k], lsem[k],
                rdests=_rdests(dq), queue_num=k,
            )

        # ---- phase B: degrees, f, g = f*x, zT partial, zw partial ----
        # Per-row dot products s = x . t directly on the PE using the
        # already-transposed x: psum_s[:, i] = sum_c xT(c,i)^T @ t_col_b[c]
        # — no 128-partition broadcast of t and no big multiply+reduce.
        tcol_b = pp.tile([P, 2], BF16)
        nc.vector.tensor_copy(tcol_b[:], ta[3][:, 0:2])
        psum_s = psw.tile([P, D], F32, tag="pw", name="psum_s")
        for i in range(T):
            for c in range(2):
                nc.tensor.matmul(
                    psum_s[:, i:i + 1],
                    lhsT=xT_all[:, c * NL + i * P:c * NL + (i + 1) * P],
                    rhs=tcol_b[:, c:c + 1],
                    start=(c == 0), stop=(c == 1),
                )
        nc.vector.tensor_copy(stl[:], psum_s[:, 0:T])
        nc.vector.tensor_mul(s_t[:], stl[:], invn[:])       # s = rowsum * invn
        nc.scalar.activation(sq_s[:], s_t[:], AF.Sqrt)
        nc.vector.reciprocal(dd[:], sq_s[:])                # d = rsqrt(s)
        nc.vector.tensor_mul(f_t[:], dd[:], invn[:])        # f = d * invn
        # Split the per-row scaling g = f*x across ACT and DVE so neither
        # engine serializes all eight tiles.
        for i in range(T):
            gs = g_all[:, i * D:(i + 1) * D]
            xs = x_all[:, i * D:(i + 1) * D]
            if i % 2 == 0:
                nc.scalar.mul(gs, xs, f_t[:, i:i + 1])
            else:
                nc.vector.tensor_scalar_mul(gs, xs, f_t[:, i:i + 1])

        psum_zT0 = psp.tile([P, D], F32, name="pzT0")
        psum_zT1 = psp.tile([P, D], F32, name="pzT1")
        for i in range(T):
            for c, pz in ((0, psum_zT0), (1, psum_zT1)):
                nc.tensor.matmul(
                    pz[:], lhsT=xb_all[:, i * D + c * P: i * D + (c + 1) * P],
                    rhs=g_all[:, i * D:(i + 1) * D],
                    start=(i == 0), stop=(i == T - 1),
                )
        for c, pz in ((0, psum_zT0), (1, psum_zT1)):
            nc.vector.tensor_copy(zT_sb[:, c * D:(c + 1) * D], pz[:])

        # zw partial = z_p @ W (fold the W GEMM before the exchange)
        for m in range(2):
            pzw = psw.tile([P, D], F32, tag="pw", name=f"pzw{m}")
            for kc in range(2):
                nc.tensor.matmul(
                    pzw[:], lhsT=zT_sb[:, kc * D + m * P: kc * D + (m + 1) * P],
                    rhs=Wb_sb[:, kc * D:(kc + 1) * D],
                    start=(kc == 0), stop=(kc == 1),
                )
            nc.vector.tensor_copy(zw_loc[:, m * D:(m + 1) * D], pzw[:])

        # ---- zw recursive doubling (hops on queues 1-3) ----
        add_z = []
        for k in range(3):
            nc.gpsimd.trigger_dma(count=None, queue_num=k,
                                  signals_writable=(za[k][:], zhr[k][:]))
            add_z.append(nc.vector.tensor_add(za[k + 1][:], za[k][:], zhr[k][:]))

        # ---- phase C: out = f * (x @ zw) ----
        for i in range(T):
            po = psw.tile([P, D], F32, tag="pw", name=f"po{i}")
            for ka in range(2):
                nc.tensor.matmul(
                    po[:], lhsT=xT_all[:, ka * NL + i * P: ka * NL + (i + 1) * P],
                    rhs=za[3][:, ka * D:(ka + 1) * D],
                    start=(ka == 0), stop=(ka == 1),
                )
            o_sb = wp.tile([P, D], F32, tag="osb", name=f"osb{i}")
            nc.scalar.mul(o_sb[:], po[:], f_t[:, i:i + 1])
            nc.sync.dma_start(out[i * P:(i + 1) * P, :], o_sb[:])

    return {"add_t": add_t, "add_z": add_z,
            "rsem_t": rsem_t, "rsem_z": rsem_z, "lsem": lsem}


def _build():
    nc = bacc.Bacc("TRN2", target_bir_lowering=False, debug=False, num_devices=R,
                   num_swdge_queues=4)
    x = nc.dram_tensor("x", [NL, D], F32, kind="ExternalInput")
    W = nc.dram_tensor("W", [D, D], F32, kind="ExternalInput")
    out = nc.dram_tensor("out", [NL, D], F32, kind="ExternalOutput")
    with tile.TileContext(nc) as tc:
        h = _program(tc, x.ap() if hasattr(x, "ap") else x, W.ap() if hasattr(W, "ap") else W, out.ap() if hasattr(out, "ap") else out)
    # Attach the cross-core waits after scheduling (the schedule-time
    # single-core sim cannot model peer sem increments, and added waits
    # only delay — they cannot invalidate the schedule).  Each hop's add
    # waits for the partner's payload (+2 on the hop's remote sem) and for
    # this core's own send of the hop to drain (+16 on the queue's local
    # sem) before overwriting the send buffer.  compile() splits
    # multi-wait instructions into event semaphores automatically.
    for k in range(3):
        h["add_t"][k].wait_op(h["rsem_t"][k], RINC[HOPS[k]], "sem-ge", check=False)
        h["add_z"][k].wait_op(h["rsem_z"][k], RINC[HOPS[k]], "sem-ge", check=False)
    nc.finalize()
    return nc


def _run(inputs, trace=False):
    if "nc" not in _cache:
        _cache["nc"] = _build()
    nc = _cache["nc"]
    x = np.ascontiguousarray(inputs["x"], dtype=np.float32)
    W = np.ascontiguousarray(inputs["W"], dtype=np.float32)
    in_maps = [{"x": x[r * NL:(r + 1) * NL], "W": W} for r in range(R)]
    res = bass_utils.run_bass_kernel_spmd(
        nc, in_maps, core_ids=list(range(R)), trace=trace,
    )
    out = np.concatenate([res.results[r]["out"] for r in range(R)], axis=0)
    return out, res


def kernel(**inputs) -> np.ndarray:
    out, _ = _run(inputs, trace=False)
    return out
